# revision 1
# baseline (speedup 1.0000x reference)
"""GRU cell kernel for Trainium2, data-parallel across 8 NeuronCores.

Per core: batch shard of 1024 rows; weights replicated.
  u  = sigmoid(x @ Wxu + h @ Whu + bu)
  r  = sigmoid(x @ Wxr + h @ Whr + br)
  c' = tanh  (x @ Wxc + (h*r) @ Whc + bc)
  c  = u*c' + (1-u)*h

Layout: all activations kept transposed in SBUF ([feature, batch]) so the
contraction dim lands on partitions; weights load in natural layout as the
stationary operand; matmuls run in float32r (full PE rate at 512-col moving).
x/h are transposed on entry and c back on exit via PE transposes.
"""

import os
import sys

import numpy as np

B = 8192
E = 1024
H = 1024
NCORES = 8
B_SH = B // NCORES  # 1024 rows per core

P = 128
KE = E // P   # 8 contraction chunks for x-side
KH = H // P   # 8 contraction chunks for h-side
NJ = H // P   # 8 output feature chunks
BN = 512      # moving free-dim per matmul (fp32 max)
NB = B_SH // BN  # 2

W_NAMES = ("Wxu", "Whu", "Wxr", "Whr", "Wxc", "Whc")
B_NAMES = ("bu", "br", "bc")

_NC_CACHE = {}


def _ensure_paths():
    for p in ("/opt/trn_rl_repo", "/root/.axon_site/_ro/trn_rl_repo"):
        if os.path.isdir(p) and p not in sys.path:
            sys.path.insert(0, p)


def _build_nc():
    import concourse.bass as bass
    import concourse.mybir as mybir
    from concourse.masks import make_identity
    from concourse.tile import TileContext

    f32 = mybir.dt.float32
    bf16 = mybir.dt.bfloat16
    AF = mybir.ActivationFunctionType

    nc = bass.Bass()
    x_d = nc.dram_tensor("input", [B_SH, E], f32, kind="ExternalInput")
    h_d = nc.dram_tensor("hidden_state", [B_SH, H], f32, kind="ExternalInput")
    w_d = {n: nc.dram_tensor(n, [E, H], f32, kind="ExternalInput") for n in W_NAMES}
    b_d = {n: nc.dram_tensor(n, [1, H], f32, kind="ExternalInput") for n in B_NAMES}
    out_d = nc.dram_tensor("output", [B_SH, H], f32, kind="ExternalOutput")

    with TileContext(nc) as tc:
        with (
            tc.tile_pool(name="sb", bufs=1) as sb,
            tc.tile_pool(name="psum", bufs=1, space="PSUM") as pp,
        ):
            ident = sb.tile([P, P], f32, tag="ident", bufs=1)
            make_identity(nc, ident[:])

            xT = [sb.tile([P, B_SH], bf16, tag=f"xT{k}", name=f"xT{k}", bufs=1) for k in range(KE)]
            hT = [sb.tile([P, B_SH], bf16, tag=f"hT{k}", name=f"hT{k}", bufs=1) for k in range(KH)]
            uT = [sb.tile([P, B_SH], f32, tag=f"uT{j}", name=f"uT{j}", bufs=1) for j in range(NJ)]
            rhT = [sb.tile([P, B_SH], bf16, tag=f"rhT{j}", name=f"rhT{j}", bufs=1) for j in range(NJ)]
            hT32 = [sb.tile([P, B_SH], f32, tag=f"hT32{k}", name=f"hT32{k}", bufs=1) for k in range(KH)]

            # ---- load x, h and transpose into [feature, batch] layout ----
            for src_d, dstT in ((x_d, xT), (h_d, hT)):
                for bi in range(B_SH // P):
                    nat = sb.tile([P, E], f32, tag="nat", bufs=3)
                    nc.sync.dma_start(nat[:], src_d[bi * P : (bi + 1) * P, :])
                    for ej in range(KE):
                        ps = pp.tile([P, P], f32, tag="pstr", bufs=4)
                        nc.tensor.transpose(
                            ps[:], nat[:, ej * P : (ej + 1) * P], ident[:]
                        )
                        nc.vector.tensor_copy(
                            dstT[ej][:, bi * P : (bi + 1) * P], ps[:]
                        )
                        if dstT is hT:
                            nc.vector.tensor_copy(
                                hT32[ej][:, bi * P : (bi + 1) * P], ps[:]
                            )

            bias_t = {}
            for g, nm in (("u", "bu"), ("r", "br"), ("c", "bc")):
                bt = sb.tile([P, NJ], f32, tag=f"bias_{g}", bufs=1)
                for j in range(NJ):
                    nc.sync.dma_start(
                        bt[:, j : j + 1],
                        b_d[nm][0:1, j * P : (j + 1) * P].rearrange("a p -> p a"),
                    )
                bias_t[g] = bt

            def dma_w_ktiles(wname):
                tiles = []
                for k in range(KE):
                    ws = sb.tile([P, E], f32, tag="wstage", name=f"ws_{wname}_{k}", bufs=3)
                    nc.sync.dma_start(ws[:], w_d[wname][k * P : (k + 1) * P, :])
                    wt = sb.tile([P, E], bf16, tag="w", name=f"w_{wname}_{k}", bufs=18)
                    nc.vector.tensor_copy(wt[:], ws[:])
                    tiles.append(wt)
                return tiles

            def gate_matmuls(ps, wxs, whs, rhsT, j, n):
                jsl = slice(j * P, (j + 1) * P)
                sl = slice(n * BN, (n + 1) * BN)
                for k in range(KE):
                    nc.tensor.matmul(
                        ps[:],
                        wxs[k][:, jsl],
                        xT[k][:, sl],
                        start=(k == 0),
                        stop=False,
                    )
                for k in range(KH):
                    nc.tensor.matmul(
                        ps[:],
                        whs[k][:, jsl],
                        rhsT[k][:, sl],
                        start=False,
                        stop=(k == KH - 1),
                    )

            # ---- gate r, then u (both sigmoid); r is folded into r*h ----
            for gname, wx, wh, dst in (("r", "Wxr", "Whr", rhT), ("u", "Wxu", "Whu", uT)):
                wxs = dma_w_ktiles(wx)
                whs = dma_w_ktiles(wh)
                for j in range(NJ):
                    for n in range(NB):
                        sl = slice(n * BN, (n + 1) * BN)
                        ps = pp.tile([P, BN], f32, tag="mm", bufs=4)
                        gate_matmuls(ps, wxs, whs, hT, j, n)
                        nc.scalar.activation(
                            dst[j][:, sl], ps[:], AF.Sigmoid,
                            bias=bias_t[gname][:, j : j + 1],
                        )
                        if gname == "r":
                            nc.vector.tensor_mul(
                                dst[j][:, sl], dst[j][:, sl], hT[j][:, sl]
                            )

            # ---- candidate + blend + output transpose, per feature chunk ----
            wxs = dma_w_ktiles("Wxc")
            whs = dma_w_ktiles("Whc")
            for j in range(NJ):
                cc = sb.tile([P, B_SH], f32, tag="cc", bufs=3)
                for n in range(NB):
                    sl = slice(n * BN, (n + 1) * BN)
                    ps = pp.tile([P, BN], f32, tag="mm", bufs=4)
                    gate_matmuls(ps, wxs, whs, rhT, j, n)
                    nc.scalar.activation(
                        cc[:, sl], ps[:], AF.Tanh, bias=bias_t["c"][:, j : j + 1]
                    )
                    # c = h + u*(c' - h), computed in place in cc
                    nc.vector.tensor_sub(cc[:, sl], cc[:, sl], hT32[j][:, sl])
                    nc.vector.tensor_mul(cc[:, sl], cc[:, sl], uT[j][:, sl])
                    nc.vector.tensor_add(cc[:, sl], cc[:, sl], hT32[j][:, sl])
                for bi in range(B_SH // P):
                    ps = pp.tile([P, P], f32, tag="pstr", bufs=4)
                    nc.tensor.transpose(
                        ps[:], cc[:, bi * P : (bi + 1) * P], ident[:]
                    )
                    ot = sb.tile([P, P], f32, tag="ost", bufs=4)
                    nc.vector.tensor_copy(ot[:], ps[:])
                    nc.sync.dma_start(
                        out_d[bi * P : (bi + 1) * P, j * P : (j + 1) * P], ot[:]
                    )

    _split_matmul_waits(nc, mybir)
    return nc


def _split_matmul_waits(nc, mybir):
    """Walrus codegen allows only one sync-wait on a Matmult (it lowers to an
    LDW+MM pair).  Spill extra waits onto a PE NoOp placed just before."""
    n_fixed = 0
    blocks = list(nc.m.functions[0].blocks)
    origs = [list(b.instructions) for b in blocks]
    spill_nops = {}  # id(inst) -> [nop insts]
    for orig in origs:
        for inst in orig:
            si = inst.sync_info
            if (
                si is not None
                and si.on_wait
                and len(si.on_wait) > 1
            ):
                waits = list(si.on_wait)
                eng = nc.engines[inst.engine]
                nops = []
                for w in waits[:-1]:
                    nop = eng.nop(hint="waitspill").ins
                    nop.sync_info = mybir.SyncInfo(on_wait=[w], on_update=[])
                    nops.append(nop)
                inst.sync_info = mybir.SyncInfo(
                    on_wait=waits[-1:], on_update=list(si.on_update or [])
                )
                spill_nops[id(inst)] = nops
                n_fixed += 1
    for blk, orig in zip(blocks, origs):
        new_list = []
        for inst in orig:
            if id(inst) in spill_nops:
                new_list.extend(spill_nops[id(inst)])
            new_list.append(inst)
        # rebuilding from `orig` also drops any freshly created nops that
        # bass appended to this block's tail
        blk.instructions[:] = new_list
    return n_fixed


def get_nc():
    if "nc" not in _NC_CACHE:
        _ensure_paths()
        _NC_CACHE["nc"] = _build_nc()
    return _NC_CACHE["nc"]


def kernel(**inputs):
    _ensure_paths()
    from concourse.bass_utils import run_bass_kernel_spmd

    nc = get_nc()

    x = np.ascontiguousarray(np.asarray(inputs["input"], dtype=np.float32))
    h = np.ascontiguousarray(np.asarray(inputs["hidden_state"], dtype=np.float32))
    shared = {
        n: np.ascontiguousarray(np.asarray(inputs[n], dtype=np.float32))
        for n in W_NAMES + B_NAMES
    }
    in_maps = []
    for c in range(NCORES):
        m = {
            "input": x[c * B_SH : (c + 1) * B_SH],
            "hidden_state": h[c * B_SH : (c + 1) * B_SH],
        }
        m.update(shared)
        in_maps.append(m)

    res = run_bass_kernel_spmd(nc, in_maps, list(range(NCORES)))
    out = np.concatenate(
        [np.asarray(res.results[c]["output"]) for c in range(NCORES)], axis=0
    )
    return out.astype(np.float32)



# revision 2
# speedup vs baseline: 1.0292x; 1.0292x over previous
"""GRU cell kernel for Trainium2, data-parallel across 8 NeuronCores.

Per core: batch shard of 1024 rows; weights replicated.
  u  = sigmoid(x @ Wxu + h @ Whu + bu)
  r  = sigmoid(x @ Wxr + h @ Whr + br)
  c' = tanh  (x @ Wxc + (h*r) @ Whc + bc)
  c  = u*c' + (1-u)*h

v2 design: the PE does ONLY the 768 matmuls (bf16, 512-col moving, ~218ns
each), everything else rides other engines:
  - host pre-casts x/h/W to bf16 (halves DMA, kills all on-chip CASTs)
  - x^T/h^T materialize via DMA XBAR transpose (dma_start_transpose)
  - r gate runs transposed (W stationary) so its bias is per-partition and
    rh^T = r^T*h^T is produced in the layout the c-gate needs as stationary
  - u and c' gates run in natural orientation (x^T/h^T/rh^T stationary,
    W moving) so the output lands [batch, H] in fp32 with no transpose;
    their free-dim biases are broadcast with a K=1 matmul and added on DVE
"""

import os
import sys

import numpy as np

B = 8192
E = 1024
H = 1024
NCORES = 8
B_SH = B // NCORES  # 1024 rows per core

P = 128
KE = E // P   # 8 contraction chunks for x-side
KH = H // P   # 8 contraction chunks for h-side
NJ = H // P   # 8 output feature chunks
BN = 512      # moving free-dim per matmul
NB = B_SH // BN  # 2

W_NAMES = ("Wxu", "Whu", "Wxr", "Whr", "Wxc", "Whc")
B_NAMES = ("bu", "br", "bc")

_NC_CACHE = {}


def _ensure_paths():
    for p in ("/opt/trn_rl_repo", "/root/.axon_site/_ro/trn_rl_repo"):
        if os.path.isdir(p) and p not in sys.path:
            sys.path.insert(0, p)


def _build_nc():
    import concourse.bass as bass
    import concourse.mybir as mybir
    from concourse.tile import TileContext

    f32 = mybir.dt.float32
    f16 = mybir.dt.float16
    bf16 = mybir.dt.bfloat16
    AF = mybir.ActivationFunctionType

    nc = bass.Bass()
    x_d = nc.dram_tensor("input", [B_SH, E], bf16, kind="ExternalInput")
    h_d = nc.dram_tensor("hidden_state", [B_SH, H], bf16, kind="ExternalInput")
    w_d = {n: nc.dram_tensor(n, [E, H], bf16, kind="ExternalInput") for n in W_NAMES}
    b_d = {n: nc.dram_tensor(n, [1, H], f32, kind="ExternalInput") for n in B_NAMES}
    out_d = nc.dram_tensor("output", [B_SH, H], f32, kind="ExternalOutput")

    with TileContext(nc) as tc:
        with (
            tc.tile_pool(name="sb", bufs=1) as sb,
            tc.tile_pool(name="psum", bufs=1, space="PSUM") as pp,
        ):
            # ---- constants / biases (tiny, issued first) ----
            ones = sb.tile([1, P], bf16, tag="ones", bufs=1)
            nc.gpsimd.memset(ones[:], 1.0)

            br_t = sb.tile([P, NJ], f32, tag="br_t", bufs=1)
            for j in range(NJ):
                nc.sync.dma_start(
                    br_t[:, j : j + 1],
                    b_d["br"][0:1, j * P : (j + 1) * P].rearrange("a p -> p a"),
                )

            brow_b = {}
            for nm in ("bu", "bc"):
                rf = sb.tile([1, H], f32, tag="brow_f", bufs=2, name=f"rf_{nm}")
                nc.sync.dma_start(rf[:], b_d[nm][0:1, :])
                rb = sb.tile([1, H], bf16, tag="brow_b", bufs=2, name=f"rb_{nm}")
                nc.vector.tensor_copy(rb[:], rf[:])
                brow_b[nm] = rb

            # ---- bulk loads: r-gate inputs first so the PE can start ----
            xT = [sb.tile([P, B_SH], bf16, tag=f"xT{k}", name=f"xT{k}", bufs=1) for k in range(KE)]
            hT = [sb.tile([P, B_SH], bf16, tag=f"hT{k}", name=f"hT{k}", bufs=1) for k in range(KH)]

            def dma_w(wname):
                tiles = []
                for k in range(KE):
                    wt = sb.tile([P, E], bf16, tag="w", name=f"w_{wname}_{k}", bufs=32)
                    nc.sync.dma_start(wt[:], w_d[wname][k * P : (k + 1) * P, :])
                    tiles.append(wt)
                return tiles

            # interleave xT transposes with Wxr so the first x-side MMs fire early
            wxr = []
            for k in range(KE):
                nc.sync.dma_start_transpose(xT[k][:], x_d[:, k * P : (k + 1) * P])
                wt = sb.tile([P, E], bf16, tag="w", name=f"w_Wxr_{k}", bufs=32)
                nc.sync.dma_start(wt[:], w_d["Wxr"][k * P : (k + 1) * P, :])
                wxr.append(wt)
            whr = []
            for k in range(KH):
                nc.sync.dma_start_transpose(hT[k][:], h_d[:, k * P : (k + 1) * P])
                wt = sb.tile([P, E], bf16, tag="w", name=f"w_Whr_{k}", bufs=32)
                nc.sync.dma_start(wt[:], w_d["Whr"][k * P : (k + 1) * P, :])
                whr.append(wt)

            # broadcast bias rows into [P, H] tiles via K=1 matmuls
            bcast = {}
            for nm in ("bu", "bc"):
                bt = sb.tile([P, H], f32, tag=f"bcast_{nm}", name=f"bcast_{nm}", bufs=1)
                for n in range(NB):
                    nsl = slice(n * BN, (n + 1) * BN)
                    ps = pp.tile([P, BN], f32, tag="mm", bufs=8, name=f"psb_{nm}{n}")
                    nc.tensor.matmul(ps[:], ones[0:1, :], brow_b[nm][0:1, nsl], start=True, stop=True)
                    nc.vector.tensor_copy(bt[:, nsl], ps[:])
                bcast[nm] = bt

            # prefetch the u-gate weights + natural-layout h during the r gate
            wxu = dma_w("Wxu")
            whu = dma_w("Whu")
            hN = []
            for b in range(B_SH // P):
                t = sb.tile([P, H], bf16, tag=f"hN{b}", name=f"hN{b}", bufs=1)
                nc.sync.dma_start(t[:], h_d[b * P : (b + 1) * P, :])
                hN.append(t)

            # ---- r gate (transposed out): rhT[j] = sigmoid(.)^T * h^T ----
            rhT = [sb.tile([P, B_SH], bf16, tag=f"rhT{j}", name=f"rhT{j}", bufs=1) for j in range(NJ)]
            for j in range(NJ):
                jsl = slice(j * P, (j + 1) * P)
                for n in range(NB):
                    nsl = slice(n * BN, (n + 1) * BN)
                    ps = pp.tile([P, BN], f32, tag="mm", bufs=8, name=f"ps_r{j}{n}")
                    for k in range(KE):
                        nc.tensor.matmul(ps[:], wxr[k][:, jsl], xT[k][:, nsl], start=(k == 0), stop=False)
                    for k in range(KH):
                        nc.tensor.matmul(ps[:], whr[k][:, jsl], hT[k][:, nsl], start=False, stop=(k == KH - 1))
                    nc.scalar.activation(rhT[j][:, nsl], ps[:], AF.Sigmoid, bias=br_t[:, j : j + 1])
                    nc.vector.tensor_mul(rhT[j][:, nsl], rhT[j][:, nsl], hT[j][:, nsl])

            # c-gate weights reuse the Wxr/Whr pool slots (WAR handled by tile deps)
            wxc = dma_w("Wxc")
            whc = dma_w("Whc")

            # ---- u gate (natural out): u[b] = sigmoid(x@Wxu + h@Whu + bu) ----
            uN = [sb.tile([P, H], f16, tag=f"uN{b}", name=f"uN{b}", bufs=1) for b in range(B_SH // P)]
            for b in range(B_SH // P):
                bsl = slice(b * P, (b + 1) * P)
                for n in range(NB):
                    nsl = slice(n * BN, (n + 1) * BN)
                    ps = pp.tile([P, BN], f32, tag="mm", bufs=8, name=f"ps_u{b}{n}")
                    for k in range(KE):
                        nc.tensor.matmul(ps[:], xT[k][:, bsl], wxu[k][:, nsl], start=(k == 0), stop=False)
                    for k in range(KH):
                        nc.tensor.matmul(ps[:], hT[k][:, bsl], whu[k][:, nsl], start=False, stop=(k == KH - 1))
                    nc.vector.tensor_add(ps[:], ps[:], bcast["bu"][:, nsl])
                    nc.scalar.activation(uN[b][:, nsl], ps[:], AF.Sigmoid)

            # ---- c gate (natural out) + blend + store ----
            for b in range(B_SH // P):
                bsl = slice(b * P, (b + 1) * P)
                for n in range(NB):
                    nsl = slice(n * BN, (n + 1) * BN)
                    ps = pp.tile([P, BN], f32, tag="mm", bufs=8, name=f"ps_c{b}{n}")
                    for k in range(KE):
                        nc.tensor.matmul(ps[:], xT[k][:, bsl], wxc[k][:, nsl], start=(k == 0), stop=False)
                    for k in range(KH):
                        nc.tensor.matmul(ps[:], rhT[k][:, bsl], whc[k][:, nsl], start=False, stop=(k == KH - 1))
                    nc.vector.tensor_add(ps[:], ps[:], bcast["bc"][:, nsl])
                    cc = sb.tile([P, BN], f32, tag="cc", bufs=4)
                    nc.scalar.activation(cc[:], ps[:], AF.Tanh)
                    # c = h + u*(c' - h)
                    nc.vector.tensor_sub(cc[:], cc[:], hN[b][:, nsl])
                    nc.vector.tensor_mul(cc[:], cc[:], uN[b][:, nsl])
                    nc.vector.tensor_add(cc[:], cc[:], hN[b][:, nsl])
                    nc.sync.dma_start(out_d[bsl, nsl], cc[:])

    _split_matmul_waits(nc, mybir)
    return nc


def _split_matmul_waits(nc, mybir):
    """Walrus codegen allows only one sync-wait on a Matmult (it lowers to an
    LDW+MM pair).  Spill extra waits onto a PE NoOp placed just before."""
    n_fixed = 0
    blocks = list(nc.m.functions[0].blocks)
    origs = [list(b.instructions) for b in blocks]
    spill_nops = {}  # id(inst) -> [nop insts]
    for orig in origs:
        for inst in orig:
            si = inst.sync_info
            if (
                si is not None
                and si.on_wait
                and len(si.on_wait) > 1
            ):
                waits = list(si.on_wait)
                eng = nc.engines[inst.engine]
                nops = []
                for w in waits[:-1]:
                    nop = eng.nop(hint="waitspill").ins
                    nop.sync_info = mybir.SyncInfo(on_wait=[w], on_update=[])
                    nops.append(nop)
                inst.sync_info = mybir.SyncInfo(
                    on_wait=waits[-1:], on_update=list(si.on_update or [])
                )
                spill_nops[id(inst)] = nops
                n_fixed += 1
    for blk, orig in zip(blocks, origs):
        new_list = []
        for inst in orig:
            if id(inst) in spill_nops:
                new_list.extend(spill_nops[id(inst)])
            new_list.append(inst)
        # rebuilding from `orig` also drops any freshly created nops that
        # bass appended to this block's tail
        blk.instructions[:] = new_list
    return n_fixed


def get_nc():
    if "nc" not in _NC_CACHE:
        _ensure_paths()
        _NC_CACHE["nc"] = _build_nc()
    return _NC_CACHE["nc"]


def make_in_maps(inputs):
    import ml_dtypes

    bf16 = ml_dtypes.bfloat16
    x = np.ascontiguousarray(np.asarray(inputs["input"], dtype=np.float32).astype(bf16))
    h = np.ascontiguousarray(np.asarray(inputs["hidden_state"], dtype=np.float32).astype(bf16))
    shared = {
        n: np.ascontiguousarray(np.asarray(inputs[n], dtype=np.float32).astype(bf16))
        for n in W_NAMES
    }
    shared.update(
        {n: np.ascontiguousarray(np.asarray(inputs[n], dtype=np.float32)) for n in B_NAMES}
    )
    in_maps = []
    for c in range(NCORES):
        m = {
            "input": x[c * B_SH : (c + 1) * B_SH],
            "hidden_state": h[c * B_SH : (c + 1) * B_SH],
        }
        m.update(shared)
        in_maps.append(m)
    return in_maps


def kernel(**inputs):
    _ensure_paths()
    from concourse.bass_utils import run_bass_kernel_spmd

    nc = get_nc()
    res = run_bass_kernel_spmd(nc, make_in_maps(inputs), list(range(NCORES)))
    out = np.concatenate(
        [np.asarray(res.results[c]["output"]) for c in range(NCORES)], axis=0
    )
    return out.astype(np.float32)


# revision 6
# speedup vs baseline: 1.4115x; 1.3714x over previous
"""GRU cell kernel for Trainium2, data-parallel across 8 NeuronCores.

Per core: batch shard of 1024 rows; weights replicated.
  u  = sigmoid(x @ Wxu + h @ Whu + bu)
  r  = sigmoid(x @ Wxr + h @ Whr + br)
  c' = tanh  (x @ Wxc + (h*r) @ Whc + bc)
  c  = u*c' + (1-u)*h

v3 design: the PE does ONLY the 768 matmuls (bf16, 512-col moving, ~218ns
each), everything else rides other engines:
  - host pre-casts to bf16 AND pre-transposes x/h (both free off-device), so
    every DRAM tensor loads with contiguous 2KB-row descriptors at full DMA
    rate (XBAR dma_start_transpose shreds into 256B descriptors - avoid)
  - r gate runs transposed (W stationary) so its bias is per-partition and
    rh^T = r^T*h^T is produced in the layout the c-gate needs as stationary
  - u and c' gates run in natural orientation (x^T/h^T/rh^T stationary,
    W moving) so the output lands [batch, H] in fp32 with no transpose;
    their free-dim biases are broadcast with a K=1 matmul and added on DVE
"""

import os
import sys

import numpy as np

B = 8192
E = 1024
H = 1024
NCORES = 8
B_SH = B // NCORES  # 1024 rows per core

P = 128
KE = E // P   # 8 contraction chunks for x-side
KH = H // P   # 8 contraction chunks for h-side
NJ = H // P   # 8 output feature chunks
BN = 512      # moving free-dim per matmul
NB = B_SH // BN  # 2

W_NAMES = ("Wxu", "Whu", "Wxr", "Whr", "Wxc", "Whc")
B_NAMES = ("bu", "br", "bc")

_NC_CACHE = {}


def _ensure_paths():
    for p in ("/opt/trn_rl_repo", "/root/.axon_site/_ro/trn_rl_repo"):
        if os.path.isdir(p) and p not in sys.path:
            sys.path.insert(0, p)


def _build_nc():
    import concourse.bass as bass
    import concourse.mybir as mybir
    from concourse.tile import TileContext

    f32 = mybir.dt.float32
    f16 = mybir.dt.float16
    bf16 = mybir.dt.bfloat16
    AF = mybir.ActivationFunctionType

    nc = bass.Bass()
    xT_d = nc.dram_tensor("inputT", [E, B_SH], bf16, kind="ExternalInput")
    hT_d = nc.dram_tensor("hiddenT", [H, B_SH], bf16, kind="ExternalInput")
    h_d = nc.dram_tensor("hidden_state", [B_SH, H], bf16, kind="ExternalInput")
    w_d = {n: nc.dram_tensor(n, [E, H], bf16, kind="ExternalInput") for n in W_NAMES}
    b_d = {n: nc.dram_tensor(n, [1, H], f32, kind="ExternalInput") for n in B_NAMES}
    out_d = nc.dram_tensor("output", [B_SH, H], f32, kind="ExternalOutput")

    with TileContext(nc) as tc:
        with (
            tc.tile_pool(name="sb", bufs=1) as sb,
            tc.tile_pool(name="psum", bufs=1, space="PSUM") as pp,
        ):
            # ---- constants / biases (tiny, issued first) ----
            ones = sb.tile([1, P], bf16, tag="ones", bufs=1)
            nc.gpsimd.memset(ones[:], 1.0)

            br_t = sb.tile([P, NJ], f32, tag="br_t", bufs=1)
            for j in range(NJ):
                nc.sync.dma_start(
                    br_t[:, j : j + 1],
                    b_d["br"][0:1, j * P : (j + 1) * P].rearrange("a p -> p a"),
                )

            brow_b = {}
            for nm in ("bu", "bc"):
                rf = sb.tile([1, H], f32, tag="brow_f", bufs=2, name=f"rf_{nm}")
                nc.sync.dma_start(rf[:], b_d[nm][0:1, :])
                rb = sb.tile([1, H], bf16, tag="brow_b", bufs=2, name=f"rb_{nm}")
                nc.vector.tensor_copy(rb[:], rf[:])
                brow_b[nm] = rb

            # ---- bulk loads: r-gate inputs first so the PE can start ----
            xT = [sb.tile([P, B_SH], bf16, tag=f"xT{k}", name=f"xT{k}", bufs=1) for k in range(KE)]
            hT = [sb.tile([P, B_SH], bf16, tag=f"hT{k}", name=f"hT{k}", bufs=1) for k in range(KH)]

            def dma_w(wname):
                tiles = []
                for k in range(KE):
                    wt = sb.tile([P, E], bf16, tag="w", name=f"w_{wname}_{k}", bufs=32)
                    nc.sync.dma_start(wt[:], w_d[wname][k * P : (k + 1) * P, :])
                    tiles.append(wt)
                return tiles

            # interleave xT loads with Wxr so the first x-side MMs fire early
            wxr = []
            for k in range(KE):
                nc.sync.dma_start(xT[k][:], xT_d[k * P : (k + 1) * P, :])
                wt = sb.tile([P, E], bf16, tag="w", name=f"w_Wxr_{k}", bufs=32)
                nc.sync.dma_start(wt[:], w_d["Wxr"][k * P : (k + 1) * P, :])
                wxr.append(wt)
            whr = []
            for k in range(KH):
                nc.sync.dma_start(hT[k][:], hT_d[k * P : (k + 1) * P, :])
                wt = sb.tile([P, E], bf16, tag="w", name=f"w_Whr_{k}", bufs=32)
                nc.sync.dma_start(wt[:], w_d["Whr"][k * P : (k + 1) * P, :])
                whr.append(wt)

            # broadcast bias rows into [P, H] tiles via K=1 matmuls
            bcast = {}
            for nm in ("bu", "bc"):
                bt = sb.tile([P, H], f32, tag=f"bcast_{nm}", name=f"bcast_{nm}", bufs=1)
                for n in range(NB):
                    nsl = slice(n * BN, (n + 1) * BN)
                    ps = pp.tile([P, BN], f32, tag="mm", bufs=8, name=f"psb_{nm}{n}")
                    nc.tensor.matmul(ps[:], ones[0:1, :], brow_b[nm][0:1, nsl], start=True, stop=True)
                    nc.vector.tensor_copy(bt[:, nsl], ps[:])
                bcast[nm] = bt

            # prefetch the u-gate weights + natural-layout h during the r gate
            wxu = dma_w("Wxu")
            whu = dma_w("Whu")
            hN = []
            for b in range(B_SH // P):
                t = sb.tile([P, H], bf16, tag=f"hN{b}", name=f"hN{b}", bufs=1)
                nc.sync.dma_start(t[:], h_d[b * P : (b + 1) * P, :])
                hN.append(t)

            # ---- r gate (transposed out): rhT[j] = sigmoid(.)^T * h^T ----
            rhT = [sb.tile([P, B_SH], bf16, tag=f"rhT{j}", name=f"rhT{j}", bufs=1) for j in range(NJ)]
            for j in range(NJ):
                jsl = slice(j * P, (j + 1) * P)
                for n in range(NB):
                    nsl = slice(n * BN, (n + 1) * BN)
                    ps = pp.tile([P, BN], f32, tag="mm", bufs=8, name=f"ps_r{j}{n}")
                    for k in range(KE):
                        nc.tensor.matmul(ps[:], wxr[k][:, jsl], xT[k][:, nsl], start=(k == 0), stop=False)
                    for k in range(KH):
                        nc.tensor.matmul(ps[:], whr[k][:, jsl], hT[k][:, nsl], start=False, stop=(k == KH - 1))
                    nc.scalar.activation(rhT[j][:, nsl], ps[:], AF.Sigmoid, bias=br_t[:, j : j + 1])
                    nc.vector.tensor_mul(rhT[j][:, nsl], rhT[j][:, nsl], hT[j][:, nsl])

            # c-gate weights reuse the Wxr/Whr pool slots (WAR handled by tile deps)
            wxc = dma_w("Wxc")
            whc = dma_w("Whc")

            # ---- u gate (natural out): u[b] = sigmoid(x@Wxu + h@Whu + bu) ----
            uN = [sb.tile([P, H], f16, tag=f"uN{b}", name=f"uN{b}", bufs=1) for b in range(B_SH // P)]
            for b in range(B_SH // P):
                bsl = slice(b * P, (b + 1) * P)
                for n in range(NB):
                    nsl = slice(n * BN, (n + 1) * BN)
                    ps = pp.tile([P, BN], f32, tag="mm", bufs=8, name=f"ps_u{b}{n}")
                    for k in range(KE):
                        nc.tensor.matmul(ps[:], xT[k][:, bsl], wxu[k][:, nsl], start=(k == 0), stop=False)
                    for k in range(KH):
                        nc.tensor.matmul(ps[:], hT[k][:, bsl], whu[k][:, nsl], start=False, stop=(k == KH - 1))
                    nc.vector.tensor_add(ps[:], ps[:], bcast["bu"][:, nsl])
                    nc.scalar.activation(uN[b][:, nsl], ps[:], AF.Sigmoid)

            # ---- c gate (natural out) + blend + store ----
            for b in range(B_SH // P):
                bsl = slice(b * P, (b + 1) * P)
                for n in range(NB):
                    nsl = slice(n * BN, (n + 1) * BN)
                    ps = pp.tile([P, BN], f32, tag="mm", bufs=8, name=f"ps_c{b}{n}")
                    for k in range(KE):
                        nc.tensor.matmul(ps[:], xT[k][:, bsl], wxc[k][:, nsl], start=(k == 0), stop=False)
                    for k in range(KH):
                        nc.tensor.matmul(ps[:], rhT[k][:, bsl], whc[k][:, nsl], start=False, stop=(k == KH - 1))
                    nc.vector.tensor_add(ps[:], ps[:], bcast["bc"][:, nsl])
                    cc = sb.tile([P, BN], f32, tag="cc", bufs=4)
                    nc.scalar.activation(cc[:], ps[:], AF.Tanh)
                    # c = h + u*(c' - h)
                    nc.vector.tensor_sub(cc[:], cc[:], hN[b][:, nsl])
                    nc.vector.tensor_mul(cc[:], cc[:], uN[b][:, nsl])
                    nc.vector.tensor_add(cc[:], cc[:], hN[b][:, nsl])
                    nc.sync.dma_start(out_d[bsl, nsl], cc[:])

    _split_matmul_waits(nc, mybir)
    return nc


def _split_matmul_waits(nc, mybir):
    """Walrus codegen allows only one sync-wait on a Matmult (it lowers to an
    LDW+MM pair).  Spill extra waits onto a PE NoOp placed just before."""
    n_fixed = 0
    blocks = list(nc.m.functions[0].blocks)
    origs = [list(b.instructions) for b in blocks]
    spill_nops = {}  # id(inst) -> [nop insts]
    for orig in origs:
        for inst in orig:
            si = inst.sync_info
            if (
                si is not None
                and si.on_wait
                and len(si.on_wait) > 1
            ):
                waits = list(si.on_wait)
                eng = nc.engines[inst.engine]
                nops = []
                for w in waits[:-1]:
                    nop = eng.nop(hint="waitspill").ins
                    nop.sync_info = mybir.SyncInfo(on_wait=[w], on_update=[])
                    nops.append(nop)
                inst.sync_info = mybir.SyncInfo(
                    on_wait=waits[-1:], on_update=list(si.on_update or [])
                )
                spill_nops[id(inst)] = nops
                n_fixed += 1
    for blk, orig in zip(blocks, origs):
        new_list = []
        for inst in orig:
            if id(inst) in spill_nops:
                new_list.extend(spill_nops[id(inst)])
            new_list.append(inst)
        # rebuilding from `orig` also drops any freshly created nops that
        # bass appended to this block's tail
        blk.instructions[:] = new_list
    return n_fixed


def get_nc():
    if "nc" not in _NC_CACHE:
        _ensure_paths()
        _NC_CACHE["nc"] = _build_nc()
    return _NC_CACHE["nc"]


def make_in_maps(inputs):
    import ml_dtypes

    bf16 = ml_dtypes.bfloat16
    x = np.asarray(inputs["input"], dtype=np.float32).astype(bf16)
    h = np.asarray(inputs["hidden_state"], dtype=np.float32).astype(bf16)
    xT = x.T  # [E, B]
    hT = h.T
    shared = {
        n: np.ascontiguousarray(np.asarray(inputs[n], dtype=np.float32).astype(bf16))
        for n in W_NAMES
    }
    shared.update(
        {n: np.ascontiguousarray(np.asarray(inputs[n], dtype=np.float32)) for n in B_NAMES}
    )
    in_maps = []
    for c in range(NCORES):
        sl = slice(c * B_SH, (c + 1) * B_SH)
        m = {
            "inputT": np.ascontiguousarray(xT[:, sl]),
            "hiddenT": np.ascontiguousarray(hT[:, sl]),
            "hidden_state": np.ascontiguousarray(h[sl]),
        }
        m.update(shared)
        in_maps.append(m)
    return in_maps


def kernel(**inputs):
    _ensure_paths()
    from concourse.bass_utils import run_bass_kernel_spmd

    nc = get_nc()
    res = run_bass_kernel_spmd(nc, make_in_maps(inputs), list(range(NCORES)))
    out = np.concatenate(
        [np.asarray(res.results[c]["output"]) for c in range(NCORES)], axis=0
    )
    return out.astype(np.float32)
